# revision 19
# baseline (speedup 1.0000x reference)
"""DendriNet Trainium2 kernel (v2: fp8 DoubleRow main, slim prep).

Computation (see reference): 3 branch layers, each doing
  h = (exc + cur) / (exc + 1 + sum_cond + inh_term)
with exc = x @ Wexc.T, inh_term = inh @ Winh.T, and W* = top32-masked exp(pre_w),
followed by a soma nonlinearity  rate = exp(la) * relu(v - sigmoid(vth))^2.

Strategy (8 NeuronCores):
  Launch 1 (tensor-parallel over weight rows, 672 rows/core):
    - per-row exact top-32 threshold via 4 rounds of DVE max8 (3 match_replace
      knockouts); mask = (exp(p+2) >= exp(t32+2)) applied on GPSIMD
    - weights stored as exp(pre_w + 2) -- lands the values in [0.9, 1.0],
      the finest region of the e4m3 grid; the e^2 factor cancels exactly in
      the h = num/den ratios by replacing the "+1" constants with "+e^2"
    - W written out untransposed in fp8e4; NO on-device transposes
  Host: relayout only -- transposes W shards into W.T [4096, 5376] (paired
    exc/inh column chunks) and x/inh into x.T column shards per core.
  Launch 2 (data-parallel over batch, 512 rows/core):
    - casts x.T/inh.T f32 -> fp8e4 on ACT
    - fp8 DoubleRow matmuls (2 contraction tiles per instruction, 2x PE rate)
    - branch-tree block sums as tiny bf16 block-diagonal matmuls (S matrices
      scaled by e^2 on device)
    - combine (reciprocal etc.) on DVE, soma nonlinearity on ACT/DVE
    - returns rate.T [128, 512] f32 per core; host transposes/concats.
"""

import math
import os
import sys

for _p in ("/opt/trn_rl_repo",):
    if os.path.isdir(_p) and _p not in sys.path:
        sys.path.insert(0, _p)

import numpy as np
import ml_dtypes

import concourse.bass as bass
import concourse.tile as tile
from concourse import bacc, mybir
from concourse.bass_utils import run_bass_kernel_spmd
from concourse.masks import make_identity

BF16 = ml_dtypes.bfloat16
F32 = np.float32

NCORES = 8
B = 4096
D = 4096
BS = B // NCORES          # 512 batch rows per core
K = 32                    # top-k per weight row

# weight tables: (rows, per-core rows)
O0, O1, OS = 2048, 512, 128
PC0, PC1, PCS = O0 // NCORES, O1 // NCORES, OS // NCORES   # 256, 64, 16
ROWS_PC = 2 * (PC0 + PC1 + PCS)                            # 672
ROWS_PAD = 768                                             # 6 tiles of 128

# global W.T column layout: paired exc/inh chunks
#  q0..q3: [e0 512 | i0 512] ; q4: [e1 512 | i1 512] ; q5: [es 128 | is 128]
PAIR_W = [512, 512, 512, 512, 512, 128]
PAIR_BASE = [0, 1024, 2048, 3072, 4096, 5120]
WT_COLS = 5376

FP_MIN = -1e30
E2 = float(math.exp(2.0))   # weights are stored as exp(pre_w + 2)
DT = mybir.dt
FP8 = ml_dtypes.float8_e4m3

LAST_PROFILE = {}


# ----------------------------------------------------------------- launch 1

def _new_nc():
    return bacc.Bacc(
        "TRN2", target_bir_lowering=False, debug=False, num_devices=NCORES)


def build_prep_kernel():
    nc = _new_nc()
    prew = nc.dram_tensor("prew", [ROWS_PAD, D], DT.float32, kind="ExternalInput")
    xtf = nc.dram_tensor("xtf", [D, BS], DT.float32, kind="ExternalInput")
    itf = nc.dram_tensor("itf", [D, BS], DT.float32, kind="ExternalInput")
    wq = nc.dram_tensor("wq", [ROWS_PAD, D], DT.float8e4, kind="ExternalOutput")
    xq = nc.dram_tensor("xq", [D, BS], DT.float8e4, kind="ExternalOutput")
    iq = nc.dram_tensor("iq", [D, BS], DT.float8e4, kind="ExternalOutput")

    xtf_v = xtf.rearrange("(a p) c -> p a c", p=128)   # [128, 32, 512] f32
    itf_v = itf.rearrange("(a p) c -> p a c", p=128)
    xq_v = xq.rearrange("(a p) c -> p a c", p=128)     # [128, 32, 512] fp8
    iq_v = iq.rearrange("(a p) c -> p a c", p=128)

    with tile.TileContext(nc) as tc:
        with (
            tc.tile_pool(name="big", bufs=2) as big,
            tc.tile_pool(name="consts", bufs=1) as consts,
            tc.tile_pool(name="small", bufs=4) as small,
            tc.tile_pool(name="xstage", bufs=2) as xstage,
        ):
            two = consts.tile([128, 1], DT.float32)
            nc.gpsimd.memset(two, 2.0)

            # x/inh f32 -> fp8 cast jobs (ACT + DMA; interleaved between
            # the DVE-bound weight tiles)
            xjobs = [(src_v, dst_v, g)
                     for src_v, dst_v in ((xtf_v, xq_v), (itf_v, iq_v))
                     for g in range(8)]

            def emit_xjob(src_v, dst_v, g):
                stg = xstage.tile([128, 4, BS], DT.float32, tag="stg")
                nc.sync.dma_start(out=stg, in_=src_v[:, 4 * g:4 * g + 4, :])
                q8 = xstage.tile([128, 4, BS], DT.float8e4, tag="q8")
                nc.scalar.activation(
                    q8, stg, mybir.ActivationFunctionType.Copy)
                nc.sync.dma_start(out=dst_v[:, 4 * g:4 * g + 4, :], in_=q8)

            xj = 0
            for t in range(ROWS_PAD // 128):
                p = big.tile([128, D], DT.float32, tag="p")
                nc.sync.dma_start(out=p, in_=prew[128 * t:128 * (t + 1), :])
                ex = big.tile([128, D], DT.float32, tag="ex")
                nc.scalar.activation(
                    ex, p, mybir.ActivationFunctionType.Exp, bias=two)
                m8 = None
                for r in range(K // 8):
                    m8 = small.tile([128, 8], DT.float32, tag=f"m8_{r}")
                    nc.vector.max(out=m8, in_=p)
                    if r < K // 8 - 1:
                        nc.vector.match_replace(
                            out=p, in_to_replace=m8, in_values=p,
                            imm_value=FP_MIN)
                # 32nd-largest value -> same exp transform as ex, so the
                # mask compare below reproduces the exact top-32 set
                t32e = small.tile([128, 1], DT.float32, tag="t32e")
                nc.scalar.activation(
                    t32e, m8[:, 7:8], mybir.ActivationFunctionType.Exp,
                    bias=two)
                w = big.tile([128, D], DT.bfloat16, tag="w")
                nc.vector.scalar_tensor_tensor(
                    out=w, in0=ex, scalar=t32e, in1=ex,
                    op0=mybir.AluOpType.is_ge, op1=mybir.AluOpType.mult)
                w8 = big.tile([128, D], DT.float8e4, tag="w8")
                nc.scalar.activation(
                    w8, w, mybir.ActivationFunctionType.Copy)
                nc.sync.dma_start(out=wq[128 * t:128 * (t + 1), :], in_=w8)
                want = (t + 1) * len(xjobs) // (ROWS_PAD // 128)
                while xj < want:
                    emit_xjob(*xjobs[xj])
                    xj += 1
    nc.compile()
    return nc


# ----------------------------------------------------------------- launch 2

def build_main_kernel():
    nc = _new_nc()
    wt = nc.dram_tensor("wt", [D, WT_COLS], DT.float8e4, kind="ExternalInput")
    xq = nc.dram_tensor("xq", [D, BS], DT.float8e4, kind="ExternalInput")
    iq = nc.dram_tensor("iq", [D, BS], DT.float8e4, kind="ExternalInput")
    s1 = nc.dram_tensor("s1", [16, 128, 128], DT.bfloat16, kind="ExternalInput")
    ss = nc.dram_tensor("ss", [4, 128, 128], DT.bfloat16, kind="ExternalInput")
    bw1 = nc.dram_tensor("bw1", [O1, 4], DT.float32, kind="ExternalInput")
    bws = nc.dram_tensor("bws", [OS, 4], DT.float32, kind="ExternalInput")
    vthp = nc.dram_tensor("vthp", [OS, 1], DT.float32, kind="ExternalInput")
    lam = nc.dram_tensor("lam", [OS, 1], DT.float32, kind="ExternalInput")
    out = nc.dram_tensor("rate", [OS, BS], DT.float32, kind="ExternalOutput")

    wt_v = wt.rearrange("(a p) c -> p a c", p=128)    # [128, 32, 5376]
    xq_v = xq.rearrange("(a p) c -> p a c", p=128)    # [128, 32, 512] fp8
    iq_v = iq.rearrange("(a p) c -> p a c", p=128)

    ND = D // 128   # 32 contraction tiles
    ND2 = ND // 2   # 16 DoubleRow steps

    with tile.TileContext(nc) as tc:
        with (
            tc.tile_pool(name="resident", bufs=1) as res,
            tc.tile_pool(name="wchunks", bufs=4) as wch,
            tc.tile_pool(name="h0p", bufs=1) as h0p,
            tc.tile_pool(name="h1p", bufs=1) as h1p,
            tc.tile_pool(name="cmb", bufs=2) as cmb,
            tc.tile_pool(name="mm", bufs=1, space="PSUM") as mm,
        ):
            xt_sb = res.tile([128, ND, BS], DT.float8e4)
            it_sb = res.tile([128, ND, BS], DT.float8e4)

            # fp8 x/inh loads, interleaved so the q0 matmuls can start
            # after the first group of each
            for g in range(8):
                nc.sync.dma_start(
                    out=xt_sb[:, 4 * g:4 * g + 4, :],
                    in_=xq_v[:, 4 * g:4 * g + 4, :])
                nc.sync.dma_start(
                    out=it_sb[:, 4 * g:4 * g + 4, :],
                    in_=iq_v[:, 4 * g:4 * g + 4, :])

            def load_consts():
                s1_sb = res.tile([128, 16, 128], DT.bfloat16, name="s1_sb")
                nc.sync.dma_start(
                    out=s1_sb, in_=s1.rearrange("k p c -> p k c"))
                ss_sb = res.tile([128, 4, 128], DT.bfloat16, name="ss_sb")
                nc.sync.dma_start(
                    out=ss_sb, in_=ss.rearrange("k p c -> p k c"))
                # scale the block matrices by e^2 (weights carry 1/e^2)
                nc.vector.tensor_scalar_mul(s1_sb, s1_sb, E2)
                nc.vector.tensor_scalar_mul(ss_sb, ss_sb, E2)

                # per-partition scalars for the q4/q5 combines:
                # e^2 * (1 + sum_conductances)
                bw1_sb = res.tile([128, 4, 4], DT.float32, name="bw1_sb")
                nc.sync.dma_start(
                    out=bw1_sb, in_=bw1.rearrange("(a p) c -> p a c", p=128))
                sw1p1 = res.tile([128, 4], DT.float32, name="sw1p1")
                for a in range(4):
                    nc.vector.tensor_reduce(
                        out=sw1p1[:, a:a + 1], in_=bw1_sb[:, a, :],
                        axis=mybir.AxisListType.X, op=mybir.AluOpType.add)
                nc.vector.tensor_scalar(
                    sw1p1, sw1p1, 1.0, E2,
                    op0=mybir.AluOpType.add, op1=mybir.AluOpType.mult)

                bws_sb = res.tile([128, 4], DT.float32, name="bws_sb")
                nc.sync.dma_start(out=bws_sb, in_=bws[:, :])
                swsp1 = res.tile([128, 1], DT.float32, name="swsp1")
                nc.vector.tensor_reduce(
                    out=swsp1, in_=bws_sb, axis=mybir.AxisListType.X,
                    op=mybir.AluOpType.add)
                nc.vector.tensor_scalar(
                    swsp1, swsp1, 1.0, E2,
                    op0=mybir.AluOpType.add, op1=mybir.AluOpType.mult)

                vth = res.tile([128, 1], DT.float32, name="vth")
                vthp_sb = res.tile([128, 1], DT.float32, name="vthp_sb")
                nc.sync.dma_start(out=vthp_sb, in_=vthp[:, :])
                nc.scalar.activation(
                    vth, vthp_sb, mybir.ActivationFunctionType.Sigmoid)
                alpha = res.tile([128, 1], DT.float32, name="alpha")
                lam_sb = res.tile([128, 1], DT.float32, name="lam_sb")
                nc.sync.dma_start(out=lam_sb, in_=lam[:, :])
                nc.scalar.activation(
                    alpha, lam_sb, mybir.ActivationFunctionType.Exp)
                return s1_sb, ss_sb, sw1p1, swsp1, vth, alpha

            consts = None

            h0t = [h0p.tile([128, BS], DT.bfloat16, tag=f"h0_{k}", name=f"h0_{k}")
                   for k in range(16)]
            h1t = [h1p.tile([128, BS], DT.bfloat16, tag=f"h1_{k}", name=f"h1_{k}")
                   for k in range(4)]

            for q in range(6):
                if q == 4 and consts is None:
                    consts = load_consts()
                    s1_sb, ss_sb, sw1p1, swsp1, vth, alpha = consts
                w = PAIR_W[q]
                nt = w // 128
                base = PAIR_BASE[q]
                pse = [mm.tile([128, BS], DT.float32, tag=f"e{ot}", name=f"pse{q}_{ot}")
                       for ot in range(nt)]
                psi = [mm.tile([128, BS], DT.float32, tag=f"i{ot}", name=f"psi{q}_{ot}")
                       for ot in range(nt)]
                for d2 in range(ND2):
                    ch = wch.tile([128, 2, 2 * w], DT.float8e4, tag="wtchunk")
                    nc.sync.dma_start(
                        out=ch,
                        in_=wt_v[:, 2 * d2:2 * d2 + 2, base:base + 2 * w])
                    st, sp = (d2 == 0), (d2 == ND2 - 1)
                    for ot in range(nt):
                        nc.tensor.matmul(
                            pse[ot], ch[:, :, 128 * ot:128 * (ot + 1)],
                            xt_sb[:, 2 * d2:2 * d2 + 2, :], start=st, stop=sp,
                            perf_mode=mybir.MatmulPerfMode.DoubleRow)
                        nc.tensor.matmul(
                            psi[ot], ch[:, :, w + 128 * ot:w + 128 * (ot + 1)],
                            it_sb[:, 2 * d2:2 * d2 + 2, :], start=st, stop=sp,
                            perf_mode=mybir.MatmulPerfMode.DoubleRow)

                if q < 4:
                    # h0 = pse / (pse + e^2 + psi); den reads the psi bank
                    # directly, esb copies pse out (ACT), freeing both banks
                    for ot in range(nt):
                        esb = cmb.tile([128, BS], DT.float32, tag="esb")
                        nc.scalar.activation(
                            esb, pse[ot], mybir.ActivationFunctionType.Copy)
                        den = cmb.tile([128, BS], DT.float32, tag="den")
                        nc.vector.scalar_tensor_tensor(
                            out=den, in0=esb, scalar=E2,
                            op0=mybir.AluOpType.add, op1=mybir.AluOpType.add,
                            in1=psi[ot])
                        rec = cmb.tile([128, BS], DT.float32, tag="rec")
                        nc.vector.reciprocal(rec, den)
                        nc.vector.tensor_mul(h0t[4 * q + ot], esb, rec)
                elif q == 4:
                    for ot in range(nt):
                        esb = cmb.tile([128, BS], DT.float32, tag="esb")
                        nc.scalar.activation(
                            esb, pse[ot], mybir.ActivationFunctionType.Copy)
                        den = cmb.tile([128, BS], DT.float32, tag="den4")
                        nc.vector.scalar_tensor_tensor(
                            out=den, in0=esb, scalar=sw1p1[:, ot:ot + 1],
                            op0=mybir.AluOpType.add, op1=mybir.AluOpType.add,
                            in1=psi[ot])
                        # cur1 = S.T @ h0 (block-diagonal, carries e^2*block_w1)
                        cur = mm.tile([128, BS], DT.float32, tag=f"e{ot}")
                        for m in range(4):
                            kk = 4 * ot + m
                            nc.tensor.matmul(
                                cur, s1_sb[:, kk, :], h0t[kk],
                                start=(m == 0), stop=(m == 3))
                        num = cmb.tile([128, BS], DT.float32, tag="num4")
                        nc.vector.tensor_add(num, esb, cur)
                        rec = cmb.tile([128, BS], DT.float32, tag="rec4")
                        nc.vector.reciprocal(rec, den)
                        nc.vector.tensor_mul(h1t[ot], num, rec)
                else:
                    esb = cmb.tile([128, BS], DT.float32, tag="esbs")
                    nc.scalar.activation(
                        esb, pse[0], mybir.ActivationFunctionType.Copy)
                    den = cmb.tile([128, BS], DT.float32, tag="dens")
                    nc.vector.scalar_tensor_tensor(
                        out=den, in0=esb, scalar=swsp1,
                        op0=mybir.AluOpType.add, op1=mybir.AluOpType.add,
                        in1=psi[0])
                    cur = mm.tile([128, BS], DT.float32, tag="e0")
                    for m in range(4):
                        nc.tensor.matmul(
                            cur, ss_sb[:, m, :], h1t[m],
                            start=(m == 0), stop=(m == 3))
                    num = cmb.tile([128, BS], DT.float32, tag="nums")
                    nc.vector.tensor_add(num, esb, cur)
                    rec = cmb.tile([128, BS], DT.float32, tag="recs")
                    nc.vector.reciprocal(rec, den)
                    v = cmb.tile([128, BS], DT.float32, tag="v")
                    nc.vector.tensor_mul(v, num, rec)
                    vd = cmb.tile([128, BS], DT.float32, tag="vd")
                    nc.vector.tensor_scalar(
                        vd, v, vth, None, op0=mybir.AluOpType.subtract)
                    rr = cmb.tile([128, BS], DT.float32, tag="rr")
                    nc.scalar.activation(
                        rr, vd, mybir.ActivationFunctionType.Relu)
                    sq = cmb.tile([128, BS], DT.float32, tag="sq")
                    nc.vector.tensor_mul(sq, rr, rr)
                    rate = cmb.tile([128, BS], DT.float32, tag="rate")
                    nc.vector.tensor_scalar(
                        rate, sq, alpha, None, op0=mybir.AluOpType.mult)
                    nc.sync.dma_start(out=out[:, :], in_=rate)
    nc.compile()
    return nc



# ----------------------------------------------------------------- fused

def build_fused_kernel():
    """Single launch: per-core top-k on 6 row-tiles, per-tile AllGather of
    W.T shards, pipelined against the data-parallel fp8 DoubleRow q-loop.

    Row assignment (host): ag[t] shard n comes from core n's tile t.
      t=0..3: cores 0-3 tile t = e0[512t+128c : +128]; cores 4-7 = i0[...]
      t=4:    cores 0-3 = e1[128c : +128];  cores 4-7 = i1[...]
      t=5:    core 0 = es, core 4 = is, others zero-pad.
    So q_t exc = ag[t] shards 0-3 (global cols in original order), inh =
    shards 4-7; q5 exc = shard 0, inh = shard 4.
    """
    nc = _new_nc()
    prew = nc.dram_tensor("prew", [ROWS_PAD, D], DT.float32, kind="ExternalInput")
    xtf = nc.dram_tensor("xtf", [D, BS], DT.float32, kind="ExternalInput")
    itf = nc.dram_tensor("itf", [D, BS], DT.float32, kind="ExternalInput")
    s1 = nc.dram_tensor("s1", [16, 128, 128], DT.bfloat16, kind="ExternalInput")
    ss = nc.dram_tensor("ss", [4, 128, 128], DT.bfloat16, kind="ExternalInput")
    bw1 = nc.dram_tensor("bw1", [O1, 4], DT.float32, kind="ExternalInput")
    bws = nc.dram_tensor("bws", [OS, 4], DT.float32, kind="ExternalInput")
    vthp = nc.dram_tensor("vthp", [OS, 1], DT.float32, kind="ExternalInput")
    lam = nc.dram_tensor("lam", [OS, 1], DT.float32, kind="ExternalInput")
    out = nc.dram_tensor("rate", [OS, BS], DT.float32, kind="ExternalOutput")

    wtloc = [nc.dram_tensor(f"wtloc{t}", [D, 128], DT.float8e4, kind="Internal")
             for t in range(6)]
    ag = [nc.dram_tensor(f"ag{t}", [NCORES * D, 128], DT.float8e4,
                         addr_space="Shared", kind="Internal")
          for t in range(6)]
    agv = [a.rearrange("(n a p) c -> p n a c", n=NCORES, p=128)
           for a in ag]
    wtloc_v = [wl.rearrange("(a p) c -> p a c", p=128) for wl in wtloc]

    xtf_v = xtf.rearrange("(a p) c -> p a c", p=128)   # [128, 32, 512] f32
    itf_v = itf.rearrange("(a p) c -> p a c", p=128)

    ND = D // 128
    ND2 = ND // 2
    NT = ROWS_PAD // 128   # 6

    with tile.TileContext(nc) as tc:
        with (
            tc.tile_pool(name="resident", bufs=1) as res,
            tc.tile_pool(name="big", bufs=2) as big,
            tc.tile_pool(name="consts", bufs=1) as consts,
            tc.tile_pool(name="small", bufs=4) as small,
            tc.tile_pool(name="xstage", bufs=2) as xstage,
            tc.tile_pool(name="wst", bufs=4) as wst,
            tc.tile_pool(name="wchunks", bufs=4) as wch,
            tc.tile_pool(name="h0p", bufs=1) as h0p,
            tc.tile_pool(name="h1p", bufs=1) as h1p,
            tc.tile_pool(name="cmb", bufs=1) as cmb,
            tc.tile_pool(name="mm", bufs=1, space="PSUM") as mm,
        ):
            two = consts.tile([128, 1], DT.float32)
            nc.gpsimd.memset(two, 2.0)
            id_bf = consts.tile([128, 128], DT.bfloat16)
            make_identity(nc, id_bf)

            xt_sb = res.tile([128, ND, BS], DT.float8e4)
            it_sb = res.tile([128, ND, BS], DT.float8e4)

            # x/inh f32 -> fp8 casts into resident SBUF (fills the window
            # while DVE runs the first top-k tiles)
            for g in range(8):
                for src_v, dst in ((xtf_v, xt_sb), (itf_v, it_sb)):
                    stg = xstage.tile([128, 4, BS], DT.float32, tag="stg")
                    nc.sync.dma_start(
                        out=stg, in_=src_v[:, 4 * g:4 * g + 4, :])
                    nc.scalar.activation(
                        dst[:, 4 * g:4 * g + 4, :], stg,
                        mybir.ActivationFunctionType.Copy)

            def load_consts():
                s1_sb = res.tile([128, 16, 128], DT.bfloat16, name="s1_sb")
                nc.sync.dma_start(
                    out=s1_sb, in_=s1.rearrange("k p c -> p k c"))
                ss_sb = res.tile([128, 4, 128], DT.bfloat16, name="ss_sb")
                nc.sync.dma_start(
                    out=ss_sb, in_=ss.rearrange("k p c -> p k c"))
                nc.vector.tensor_scalar_mul(s1_sb, s1_sb, E2)
                nc.vector.tensor_scalar_mul(ss_sb, ss_sb, E2)

                bw1_sb = res.tile([128, 4, 4], DT.float32, name="bw1_sb")
                nc.sync.dma_start(
                    out=bw1_sb, in_=bw1.rearrange("(a p) c -> p a c", p=128))
                sw1p1 = res.tile([128, 4], DT.float32, name="sw1p1")
                for a in range(4):
                    nc.vector.tensor_reduce(
                        out=sw1p1[:, a:a + 1], in_=bw1_sb[:, a, :],
                        axis=mybir.AxisListType.X, op=mybir.AluOpType.add)
                nc.vector.tensor_scalar(
                    sw1p1, sw1p1, 1.0, E2,
                    op0=mybir.AluOpType.add, op1=mybir.AluOpType.mult)

                bws_sb = res.tile([128, 4], DT.float32, name="bws_sb")
                nc.sync.dma_start(out=bws_sb, in_=bws[:, :])
                swsp1 = res.tile([128, 1], DT.float32, name="swsp1")
                nc.vector.tensor_reduce(
                    out=swsp1, in_=bws_sb, axis=mybir.AxisListType.X,
                    op=mybir.AluOpType.add)
                nc.vector.tensor_scalar(
                    swsp1, swsp1, 1.0, E2,
                    op0=mybir.AluOpType.add, op1=mybir.AluOpType.mult)

                vth = res.tile([128, 1], DT.float32, name="vth")
                vthp_sb = res.tile([128, 1], DT.float32, name="vthp_sb")
                nc.sync.dma_start(out=vthp_sb, in_=vthp[:, :])
                nc.scalar.activation(
                    vth, vthp_sb, mybir.ActivationFunctionType.Sigmoid)
                alpha = res.tile([128, 1], DT.float32, name="alpha")
                lam_sb = res.tile([128, 1], DT.float32, name="lam_sb")
                nc.sync.dma_start(out=lam_sb, in_=lam[:, :])
                nc.scalar.activation(
                    alpha, lam_sb, mybir.ActivationFunctionType.Exp)
                return s1_sb, ss_sb, sw1p1, swsp1, vth, alpha

            h0t = [h0p.tile([128, BS], DT.bfloat16, tag=f"h0_{k}",
                            name=f"h0_{k}") for k in range(16)]
            h1t = [h1p.tile([128, BS], DT.bfloat16, tag=f"h1_{k}",
                            name=f"h1_{k}") for k in range(4)]
            consts_t = [None]

            def emit_tile(t):
                # exact top-32 threshold + masked exp(p+2) -> W.T shard
                p = big.tile([128, D], DT.float32, tag="p", name="p")
                nc.sync.dma_start(out=p, in_=prew[128 * t:128 * (t + 1), :])
                ex = big.tile([128, D], DT.float32, tag="ex", name="ex")
                nc.scalar.activation(
                    ex, p, mybir.ActivationFunctionType.Exp, bias=two)
                m8 = None
                for r in range(K // 8):
                    m8 = small.tile([128, 8], DT.float32, tag=f"m8_{r}",
                                    name="m8")
                    nc.vector.max(out=m8, in_=p)
                    if r < K // 8 - 1:
                        nc.vector.match_replace(
                            out=p, in_to_replace=m8, in_values=p,
                            imm_value=FP_MIN)
                t32e = small.tile([128, 1], DT.float32, tag="t32e",
                                  name="t32e")
                nc.scalar.activation(
                    t32e, m8[:, 7:8], mybir.ActivationFunctionType.Exp,
                    bias=two)
                w = big.tile([128, D], DT.bfloat16, tag="w", name="w")
                nc.vector.scalar_tensor_tensor(
                    out=w, in0=ex, scalar=t32e, in1=ex,
                    op0=mybir.AluOpType.is_ge, op1=mybir.AluOpType.mult)
                # PE-transpose to W.T, cast to fp8 on the PSUM copy-out
                for g in range(8):
                    ps = mm.tile([128, 4, 128], DT.bfloat16, tag="i3",
                                 name="ps")
                    for j4 in range(4):
                        j = 4 * g + j4
                        # accumulate into slice j4: start zeroes the whole
                        # 2KB zero-region once, later slices add onto zeros
                        nc.tensor.matmul(
                            ps[:, j4, :], w[:, 128 * j:128 * (j + 1)], id_bf,
                            is_transpose=True, start=(j4 == 0),
                            stop=(j4 == 3), skip_group_check=True)
                    st = wst.tile([128, 4, 128], DT.float8e4, tag="st",
                                  name="st")
                    nc.scalar.activation(
                        st, ps, mybir.ActivationFunctionType.Copy)
                    nc.sync.dma_start(
                        out=wtloc_v[t][:, 4 * g:4 * g + 4, :], in_=st)

            def emit_zero_tile(t):
                st = wst.tile([128, 4, 128], DT.float8e4, tag="st", name="st")
                nc.vector.memset(st, 0.0)
                for g in range(8):
                    nc.sync.dma_start(
                        out=wtloc_v[t][:, 4 * g:4 * g + 4, :], in_=st)

            def emit_ag(t):
                nc.gpsimd.collective_compute(
                    kind="AllGather", op=mybir.AluOpType.bypass,
                    replica_groups=[list(range(NCORES))],
                    ins=[wtloc[t][:, :]], outs=[ag[t][:, :]])

            def emit_q(q):
                nt = 4 if q < 5 else 1
                pse = [mm.tile([128, BS], DT.float32, tag=f"e{ot}",
                               name=f"pse{q}_{ot}") for ot in range(nt)]
                psi = [mm.tile([128, BS], DT.float32, tag=f"i{ot}",
                               name=f"psi{q}_{ot}") for ot in range(nt)]
                for d2 in range(ND2):
                    ch = wch.tile([128, NCORES, 2, 128], DT.float8e4,
                                  tag="ch", name="ch")
                    for a2 in range(2):
                        nc.sync.dma_start(
                            out=ch[:, :, a2, :],
                            in_=agv[q][:, :, 2 * d2 + a2, :])
                    st_, sp = (d2 == 0), (d2 == ND2 - 1)
                    for ot in range(nt):
                        nc.tensor.matmul(
                            pse[ot], ch[:, ot, :, :],
                            xt_sb[:, 2 * d2:2 * d2 + 2, :],
                            start=st_, stop=sp,
                            perf_mode=mybir.MatmulPerfMode.DoubleRow)
                        nc.tensor.matmul(
                            psi[ot], ch[:, 4 + ot, :, :],
                            it_sb[:, 2 * d2:2 * d2 + 2, :],
                            start=st_, stop=sp,
                            perf_mode=mybir.MatmulPerfMode.DoubleRow)

                if q < 4:
                    # h0 = pse / (pse + e^2 + psi): ACT copies PSUM out
                    # (folding +e^2 into the psi copy), Pool adds, DVE only
                    # does the reciprocal, Pool multiplies.
                    for ot in range(nt):
                        esb = cmb.tile([128, BS], DT.float32, tag="esb",
                                       name="esb")
                        nc.scalar.activation(
                            esb, pse[ot], mybir.ActivationFunctionType.Copy)
                        isbp = cmb.tile([128, BS], DT.float32, tag="isbp",
                                        name="isbp")
                        nc.scalar.activation(
                            isbp, psi[ot], mybir.ActivationFunctionType.Copy,
                            bias=E2)
                        den = cmb.tile([128, BS], DT.float32, tag="den",
                                       name="den")
                        nc.gpsimd.tensor_tensor(
                            out=den, in0=esb, in1=isbp,
                            op=mybir.AluOpType.add)
                        rec = cmb.tile([128, BS], DT.float32, tag="rec",
                                       name="rec")
                        nc.vector.reciprocal(rec, den)
                        nc.gpsimd.tensor_tensor(
                            out=h0t[4 * q + ot], in0=esb, in1=rec,
                            op=mybir.AluOpType.mult)
                elif q == 4:
                    s1_sb, ss_sb, sw1p1, swsp1, vth, alpha = consts_t[0]
                    for ot in range(nt):
                        esb = cmb.tile([128, BS], DT.float32, tag="esb",
                                       name="esb")
                        nc.scalar.activation(
                            esb, pse[ot], mybir.ActivationFunctionType.Copy)
                        den = cmb.tile([128, BS], DT.float32, tag="den4",
                                       name="den")
                        nc.vector.scalar_tensor_tensor(
                            out=den, in0=esb, scalar=sw1p1[:, ot:ot + 1],
                            op0=mybir.AluOpType.add, op1=mybir.AluOpType.add,
                            in1=psi[ot])
                        cur = mm.tile([128, BS], DT.float32, tag=f"e{ot}",
                                      name="cur")
                        for m in range(4):
                            kk = 4 * ot + m
                            nc.tensor.matmul(
                                cur, s1_sb[:, kk, :], h0t[kk],
                                start=(m == 0), stop=(m == 3))
                        num = cmb.tile([128, BS], DT.float32, tag="num4",
                                       name="num")
                        nc.vector.tensor_add(num, esb, cur)
                        rec = cmb.tile([128, BS], DT.float32, tag="rec4",
                                       name="rec")
                        nc.vector.reciprocal(rec, den)
                        nc.vector.tensor_mul(h1t[ot], num, rec)
                else:
                    s1_sb, ss_sb, sw1p1, swsp1, vth, alpha = consts_t[0]
                    esb = cmb.tile([128, BS], DT.float32, tag="esbs",
                                   name="esb")
                    nc.scalar.activation(
                        esb, pse[0], mybir.ActivationFunctionType.Copy)
                    den = cmb.tile([128, BS], DT.float32, tag="dens",
                                   name="den")
                    nc.vector.scalar_tensor_tensor(
                        out=den, in0=esb, scalar=swsp1,
                        op0=mybir.AluOpType.add, op1=mybir.AluOpType.add,
                        in1=psi[0])
                    cur = mm.tile([128, BS], DT.float32, tag="e0", name="cur")
                    for m in range(4):
                        nc.tensor.matmul(
                            cur, ss_sb[:, m, :], h1t[m],
                            start=(m == 0), stop=(m == 3))
                    num = cmb.tile([128, BS], DT.float32, tag="nums",
                                   name="num")
                    nc.vector.tensor_add(num, esb, cur)
                    rec = cmb.tile([128, BS], DT.float32, tag="recs",
                                   name="rec")
                    nc.vector.reciprocal(rec, den)
                    v = cmb.tile([128, BS], DT.float32, tag="v", name="v")
                    nc.vector.tensor_mul(v, num, rec)
                    vd = cmb.tile([128, BS], DT.float32, tag="vd", name="vd")
                    nc.vector.tensor_scalar(
                        vd, v, vth, None, op0=mybir.AluOpType.subtract)
                    rr = cmb.tile([128, BS], DT.float32, tag="rr", name="rr")
                    nc.scalar.activation(
                        rr, vd, mybir.ActivationFunctionType.Relu)
                    sq = cmb.tile([128, BS], DT.float32, tag="sq", name="sq")
                    nc.vector.tensor_mul(sq, rr, rr)
                    rate = cmb.tile([128, BS], DT.float32, tag="rate",
                                    name="rate")
                    nc.vector.tensor_scalar(
                        rate, sq, alpha, None, op0=mybir.AluOpType.mult)
                    nc.sync.dma_start(out=out[:, :], in_=rate)

            for t in range(NT):
                if t < 5:
                    emit_tile(t)
                else:
                    # tile 5 is real only on cores 0 (es) and 4 (is); the
                    # kernel is SPMD so every core runs the full top-k on
                    # its tile-5 rows (zero rows on cores 1-3/5-7 produce
                    # unused shards of ag[5])
                    emit_tile(t)
                emit_ag(t)
                if t == NT - 1:
                    consts_t[0] = load_consts()
                if t >= 1:
                    emit_q(t - 1)
            emit_q(NT - 1)
    nc.compile()
    return nc


def kernel_fused(x, inhibitory_input, pre_w_exc0, pre_w_inh0, pre_w_exc1,
                 pre_w_inh1, block_w1, pre_w_exc_s, pre_w_inh_s, block_w_s,
                 presigmoid_Vth, log_alpha_max):
    x = np.asarray(x, F32)
    inh = np.asarray(inhibitory_input, F32)
    e0 = np.asarray(pre_w_exc0, F32)
    i0 = np.asarray(pre_w_inh0, F32)
    e1 = np.asarray(pre_w_exc1, F32)
    i1 = np.asarray(pre_w_inh1, F32)
    es = np.asarray(pre_w_exc_s, F32)
    is_ = np.asarray(pre_w_inh_s, F32)

    if "fused" not in _CACHE:
        _CACHE["fused"] = build_fused_kernel()
    trace = bool(os.environ.get("BASS_TRACE"))
    if trace:
        _install_ntff_hook()

    s1m, ssm = _build_s_mats(block_w1, block_w_s)
    bw1 = np.ascontiguousarray(np.asarray(block_w1, F32).reshape(O1, 4))
    bws = np.ascontiguousarray(np.asarray(block_w_s, F32).reshape(OS, 4))
    vthp = np.ascontiguousarray(np.asarray(presigmoid_Vth, F32).reshape(OS, 1))
    lam = np.ascontiguousarray(np.asarray(log_alpha_max, F32).reshape(OS, 1))

    xT = np.ascontiguousarray(x.T)       # layout only
    iT = np.ascontiguousarray(inh.T)

    zero128 = np.zeros((128, D), F32)
    in_maps = []
    for c in range(NCORES):
        blocks = []
        if c < 4:
            for t in range(4):
                blocks.append(e0[512 * t + 128 * c: 512 * t + 128 * (c + 1)])
            blocks.append(e1[128 * c:128 * (c + 1)])
            blocks.append(es if c == 0 else zero128)
        else:
            cp = c - 4
            for t in range(4):
                blocks.append(i0[512 * t + 128 * cp: 512 * t + 128 * (cp + 1)])
            blocks.append(i1[128 * cp:128 * (cp + 1)])
            blocks.append(is_ if cp == 0 else zero128)
        prew = np.ascontiguousarray(np.concatenate(blocks))
        in_maps.append({
            "prew": prew,
            "xtf": np.ascontiguousarray(xT[:, BS * c:BS * (c + 1)]),
            "itf": np.ascontiguousarray(iT[:, BS * c:BS * (c + 1)]),
            "s1": s1m, "ss": ssm, "bw1": bw1, "bws": bws,
            "vthp": vthp, "lam": lam,
        })
    r = run_bass_kernel_spmd(
        _CACHE["fused"], in_maps, core_ids=list(range(NCORES)), trace=trace)
    LAST_PROFILE["prep_ns"] = 0
    LAST_PROFILE["main_ns"] = r.exec_time_ns

    outp = np.empty((B, OS), F32)
    for c in range(NCORES):
        outp[BS * c:BS * (c + 1), :] = np.asarray(r.results[c]["rate"]).T
    return outp


# ----------------------------------------------------------------- host glue

def _build_s_mats(block_w1, block_w_s):
    bw1f = np.asarray(block_w1, F32).reshape(-1)       # [2048]
    bwsf = np.asarray(block_w_s, F32).reshape(-1)      # [512]
    p = np.arange(128)
    s1 = np.zeros((16, 128, 128), F32)
    for k in range(16):
        c = 32 * (k % 4) + p // 4
        s1[k, p, c] = bw1f[128 * k + p]
    ssm = np.zeros((4, 128, 128), F32)
    for m in range(4):
        c = 32 * m + p // 4
        ssm[m, p, c] = bwsf[128 * m + p]
    return s1.astype(BF16), ssm.astype(BF16)


_CACHE = {}


def _install_ntff_hook():
    """bass_utils' trace path looks up antenv.axon_hooks, which this image
    lacks; synthesize it and register the ctypes NTFF hook."""
    import types
    if "antenv.axon_hooks" in sys.modules:
        return
    try:
        from trn_agent_boot.trn_boot import _ntff_profile_via_ctypes
        hook = _ntff_profile_via_ctypes("/opt/axon/libaxon_pjrt.so")
    except Exception:
        hook = None
    mod = types.ModuleType("antenv.axon_hooks")
    _h = [hook]
    mod.set_axon_ntff_profile_hook = lambda h: _h.__setitem__(0, h)
    mod.get_axon_ntff_profile_hook = lambda: _h[0]
    sys.modules["antenv.axon_hooks"] = mod
    try:
        import antenv
        antenv.axon_hooks = mod
    except Exception:
        pass


def kernel(*args, **kw):
    if os.environ.get("KERNEL_FUSED"):
        return kernel_fused(*args, **kw)
    return kernel_two_launch(*args, **kw)


def kernel_two_launch(x, inhibitory_input, pre_w_exc0, pre_w_inh0, pre_w_exc1, pre_w_inh1,
           block_w1, pre_w_exc_s, pre_w_inh_s, block_w_s, presigmoid_Vth,
           log_alpha_max):
    x = np.asarray(x, F32)
    inh = np.asarray(inhibitory_input, F32)
    e0 = np.asarray(pre_w_exc0, F32)
    i0 = np.asarray(pre_w_inh0, F32)
    e1 = np.asarray(pre_w_exc1, F32)
    i1 = np.asarray(pre_w_inh1, F32)
    es = np.asarray(pre_w_exc_s, F32)
    is_ = np.asarray(pre_w_inh_s, F32)

    if "prep" not in _CACHE:
        _CACHE["prep"] = build_prep_kernel()
        _CACHE["main"] = build_main_kernel()
    trace = bool(os.environ.get("BASS_TRACE"))
    if trace:
        _install_ntff_hook()

    xT = np.ascontiguousarray(x.T)       # layout only
    iT = np.ascontiguousarray(inh.T)

    in_maps = []
    for c in range(NCORES):
        prew = np.concatenate([
            e0[PC0 * c:PC0 * (c + 1)], e1[PC1 * c:PC1 * (c + 1)],
            es[PCS * c:PCS * (c + 1)],
            i0[PC0 * c:PC0 * (c + 1)], i1[PC1 * c:PC1 * (c + 1)],
            is_[PCS * c:PCS * (c + 1)],
            np.zeros((ROWS_PAD - ROWS_PC, D), F32),
        ])
        in_maps.append({
            "prew": np.ascontiguousarray(prew),
            "xtf": np.ascontiguousarray(xT[:, BS * c:BS * (c + 1)]),
            "itf": np.ascontiguousarray(iT[:, BS * c:BS * (c + 1)]),
        })
    r1 = run_bass_kernel_spmd(
        _CACHE["prep"], in_maps, core_ids=list(range(NCORES)), trace=trace)
    LAST_PROFILE["prep_ns"] = r1.exec_time_ns

    # reassemble global W.T [4096, 5376] (layout only: slice + transpose)
    WT_T = np.empty((WT_COLS, D), FP8)
    for c in range(NCORES):
        wc = np.asarray(r1.results[c]["wq"])
        b0 = 1024 * (c // 2) + 256 * (c % 2)
        WT_T[b0:b0 + 256] = wc[0:256]                    # exc0
        WT_T[b0 + 512:b0 + 768] = wc[336:592]            # inh0
        WT_T[4096 + 64 * c:4096 + 64 * (c + 1)] = wc[256:320]    # exc1
        WT_T[4608 + 64 * c:4608 + 64 * (c + 1)] = wc[592:656]    # inh1
        WT_T[5120 + 16 * c:5120 + 16 * (c + 1)] = wc[320:336]    # exc_s
        WT_T[5248 + 16 * c:5248 + 16 * (c + 1)] = wc[656:672]    # inh_s
    WT = np.ascontiguousarray(WT_T.T)                    # [4096, 5376]

    s1m, ssm = _build_s_mats(block_w1, block_w_s)
    bw1 = np.ascontiguousarray(np.asarray(block_w1, F32).reshape(O1, 4))
    bws = np.ascontiguousarray(np.asarray(block_w_s, F32).reshape(OS, 4))
    vthp = np.ascontiguousarray(np.asarray(presigmoid_Vth, F32).reshape(OS, 1))
    lam = np.ascontiguousarray(np.asarray(log_alpha_max, F32).reshape(OS, 1))

    in_maps2 = []
    for c in range(NCORES):
        in_maps2.append({
            "wt": WT,
            "xq": np.asarray(r1.results[c]["xq"]),
            "iq": np.asarray(r1.results[c]["iq"]),
            "s1": s1m, "ss": ssm, "bw1": bw1, "bws": bws,
            "vthp": vthp, "lam": lam,
        })
    r2 = run_bass_kernel_spmd(
        _CACHE["main"], in_maps2, core_ids=list(range(NCORES)), trace=trace)
    LAST_PROFILE["main_ns"] = r2.exec_time_ns

    outp = np.empty((B, OS), F32)
    for c in range(NCORES):
        outp[BS * c:BS * (c + 1), :] = r2.results[c]["rate"].T
    return outp
